# revision 22
# baseline (speedup 1.0000x reference)
"""Trainium2 Bass kernel for nn_EncoderLayer (D=512, H=8, DK=64, DF=2048, B=2, S=2048).

Strategy (8 NeuronCores):
  - Attention is head-parallel: core c computes head c for both batches.
    All on-chip attention work happens in transposed layout (features on
    partitions, tokens on the free dim) so no large transposes are needed:
    the host passes x pre-transposed (xT[b] = x[b].T).
  - The softmax denominator is fused into the attention-output matmul by
    augmenting V with a ones column (out row 64 = sum of exp scores).
    Softmax max-subtraction is skipped: scores*0.125 is O(1) here, exp is
    safely in range, and exp(s)/sum(exp(s)) is exact regardless.
  - One AllToAll (1 MB/core) redistributes per-head attention outputs to
    per-token shards; each core then does Wo projection + LN1 + FFN + LN2
    for its 512 tokens and returns its output shard.
"""

import numpy as np

import concourse.bass as bass
import concourse.tile as tile
from concourse import mybir
from concourse.bass_utils import run_bass_kernel_spmd
from concourse.masks import make_identity
from concourse.vector_clock import ScopedClock

F32 = mybir.dt.float32
F32R = mybir.dt.float32r
BF16 = mybir.dt.bfloat16
AF = mybir.ActivationFunctionType
ALU = mybir.AluOpType

B, S, D, H, DK, DF = 2, 2048, 512, 8, 64, 2048
N_CORES = 8
NSH = (B * S) // N_CORES  # tokens per core shard = 512
EPS = 1e-5

# ---------------------------------------------------------------------------
# Workaround: this walrus build rejects >1 sem wait on CTRL-type (drain)
# instructions. Split the TileContext tail-drain waits onto dedicated
# single-wait nops; the all-engine barrier right after keeps semantics.


def _split_excess_waits(nc, max_waits=1):
    """Hoist sem waits beyond `max_waits` onto dedicated single-wait nops
    inserted just before the instruction on the same engine queue."""
    for fn in nc.m.functions:
        for bb in fn.blocks:
            new_list = []
            for inst in bb.instructions:
                si = inst.sync_info
                waits = list(si.on_wait) if si is not None and si.on_wait else []
                if len(waits) > max_waits:
                    keep = waits[:max_waits]
                    extra = waits[max_waits:]
                    si.on_wait = keep
                    for w in extra:
                        nop = mybir.InstNoOp(name=f"I-waitnop-{nc.next_id()}")
                        nop.engine = inst.engine
                        nop.sync_info = mybir.SyncInfo(on_wait=[w], on_update=[])
                        new_list.append(nop)
                new_list.append(inst)
            bb.instructions = new_list


def _bcast_ap(handle, parts, n):
    """AP reading a 1-D DRAM tensor broadcast across `parts` partitions."""
    a = handle[:]
    return bass.AP(tensor=a.tensor, offset=a.offset, ap=[[0, parts], [1, n]])


def build_nc():
    nc = bass.Bass(target_bir_lowering=False)

    # ---- kernel I/O (per core) ----
    xT = nc.dram_tensor("xT", [B, D, S], F32R, kind="ExternalInput")
    xsb = nc.dram_tensor("xsb", [NSH, D], F32, kind="ExternalInput")  # x_shard + bo
    wq = nc.dram_tensor("wq", [D, DK], F32R, kind="ExternalInput")
    wk = nc.dram_tensor("wk", [D, DK], F32R, kind="ExternalInput")
    wv = nc.dram_tensor("wv", [D, DK], F32R, kind="ExternalInput")
    bq = nc.dram_tensor("bq", [DK, 1], F32, kind="ExternalInput")
    bk = nc.dram_tensor("bk", [DK, 1], F32, kind="ExternalInput")
    bv = nc.dram_tensor("bv", [DK], F32, kind="ExternalInput")
    wo = nc.dram_tensor("wo", [H * DK, D], F32R, kind="ExternalInput")
    w1 = nc.dram_tensor("w1", [D, DF], F32R, kind="ExternalInput")
    b1c = nc.dram_tensor("b1c", [128, DF // 128], F32, kind="ExternalInput")
    w2 = nc.dram_tensor("w2", [DF, D], F32R, kind="ExternalInput")
    b2 = nc.dram_tensor("b2", [D], F32, kind="ExternalInput")
    g1 = nc.dram_tensor("g1", [D], F32, kind="ExternalInput")
    be1 = nc.dram_tensor("be1", [D], F32, kind="ExternalInput")
    g2 = nc.dram_tensor("g2", [D], F32, kind="ExternalInput")
    be2 = nc.dram_tensor("be2", [D], F32, kind="ExternalInput")
    out_shard = nc.dram_tensor("out_shard", [NSH, D], F32, kind="ExternalOutput")

    with tile.TileContext(nc) as tc:
        with (
            tc.tile_pool(name="consts", bufs=1) as consts,
            tc.tile_pool(name="xt", bufs=4) as xt_pool,
            tc.tile_pool(name="qk", bufs=2) as qk_pool,
            tc.tile_pool(name="vaug", bufs=2) as v_pool,
            tc.tile_pool(name="expt", bufs=3) as exp_pool,
            tc.tile_pool(name="otn", bufs=3) as ot_pool,
            tc.tile_pool(name="wff", bufs=3) as wff_pool,
            tc.tile_pool(name="f1p", bufs=3) as f1_pool,
            tc.tile_pool(name="hh", bufs=2) as h_pool,
            tc.tile_pool(name="tmps", bufs=2) as tmp_pool,
            tc.tile_pool(name="small", bufs=2) as small,
            tc.tile_pool(name="psc", bufs=2, space="PSUM") as psc,
            tc.tile_pool(name="pacc", bufs=4, space="PSUM") as pacc,
            tc.tile_pool(name="dram", bufs=1, space="DRAM") as dram,
        ):
            # ---- first: start streaming x (b=0) and the QKV weights so the
            # PE can begin as early as possible; other consts follow.
            xt_first = []
            for d in range(4):
                t_ = xt_pool.tile([128, S], F32R, tag="xt", name=f"xt0_{d}")
                nc.sync.dma_start(out=t_, in_=xT[0, 128 * d : 128 * (d + 1), :])
                xt_first.append(t_)
            wq_sb = consts.tile([128, 4 * DK], F32R, tag="wq_sb")
            wk_sb = consts.tile([128, 4 * DK], F32R, tag="wk_sb")
            wv_sb = consts.tile([128, 4 * DK], F32R, tag="wv_sb")
            for w_sb, w_h in ((wq_sb, wq), (wk_sb, wk), (wv_sb, wv)):
                nc.sync.dma_start(
                    out=w_sb[:].rearrange("p (d k) -> p d k", k=DK),
                    in_=w_h[:, :].rearrange("(d p) k -> p d k", p=128),
                )
            bq_sb = consts.tile([DK, 1], F32, tag="bq_sb")
            bk_sb = consts.tile([DK, 1], F32, tag="bk_sb")
            nc.sync.dma_start(out=bq_sb, in_=bq[:, :])
            nc.sync.dma_start(out=bk_sb, in_=bk[:, :])

            ident = consts.tile([128, 128], F32)
            make_identity(nc, ident)
            eps_t = consts.tile([128, 1], F32)
            nc.vector.memset(eps_t, EPS)
            ones1 = consts.tile([1, DK], F32R)
            nc.vector.memset(ones1[:].bitcast(F32), 1.0)
            bv_bc = consts.tile([128, DK], F32)
            nc.gpsimd.dma_start(out=bv_bc, in_=_bcast_ap(bv, 128, DK))
            bv8 = consts.tile([128, 8 * DK], F32)
            for i in range(8):
                nc.vector.tensor_copy(bv8[:, i * DK : (i + 1) * DK], bv_bc)

            wo_sb = consts.tile([128, 4 * D], F32R, tag="wo_sb")
            nc.sync.dma_start(
                out=wo_sb[:].rearrange("p (c d) -> p c d", d=D),
                in_=wo[:, :].rearrange("(c p) d -> p c d", p=128),
            )
            b1_sb = consts.tile([128, DF // 128], F32, tag="b1_sb")
            nc.sync.dma_start(out=b1_sb, in_=b1c[:, :])

            b2_t = consts.tile([128, D], F32, tag="b2_t")
            g1_t = consts.tile([128, D], F32, tag="g1_t")
            be1_t = consts.tile([128, D], F32, tag="be1_t")
            g2_t = consts.tile([128, D], F32, tag="g2_t")
            be2_t = consts.tile([128, D], F32, tag="be2_t")
            for t_sb, h_d in ((b2_t, b2), (g1_t, g1), (be1_t, be1), (g2_t, g2), (be2_t, be2)):
                nc.gpsimd.dma_start(out=t_sb, in_=_bcast_ap(h_d, 128, D))

            xsbo = []
            for i in range(4):
                xt_ = consts.tile([128, D], F32, tag=f"xsbo{i}")
                nc.sync.dma_start(out=xt_, in_=xsb[128 * i : 128 * (i + 1), :])
                xsbo.append(xt_)

            send2d = dram.tile([128, 2048], F32R)
            recv2d = dram.tile([128, 2048], F32R)

            # =========== per-batch: QKV + attention ===========
            for b in range(B):
                if b == 0:
                    xt = xt_first
                else:
                    xt = []
                    for d in range(4):
                        t_ = xt_pool.tile([128, S], F32R, tag="xt", name=f"xt1_{d}")
                        nc.sync.dma_start(out=t_, in_=xT[b, 128 * d : 128 * (d + 1), :])
                        xt.append(t_)

                # qT/kT [128, 2048]: rows 0-63 = projection, rows 64-127
                # zeroed so the scores matmul can run K=128 (fp32r K=64 MMs
                # do not register as PE activity for the HAM clock-gate and
                # run at 1.2 GHz; K=128 keeps the PE warm at 2.4 GHz).
                qT = qk_pool.tile([128, S], F32R, tag="qT")
                kT = qk_pool.tile([128, S], F32R, tag="kT")
                nc.vector.memset(qT[64:128, :].bitcast(F32), 0.0)
                nc.vector.memset(kT[64:128, :].bitcast(F32), 0.0)
                for dst, w_sb, b_sb in ((qT, wq_sb, bq_sb), (kT, wk_sb, bk_sb)):
                    for s4 in range(4):
                        ps = psc.tile([DK, 512], F32, tag="sc")
                        for d in range(4):
                            nc.tensor.matmul(
                                ps,
                                lhsT=w_sb[:, DK * d : DK * (d + 1)],
                                rhs=xt[d][:, 512 * s4 : 512 * (s4 + 1)],
                                start=(d == 0),
                                stop=(d == 3),
                            )
                        nc.scalar.activation(
                            out=dst[0:DK, 512 * s4 : 512 * (s4 + 1)],
                            in_=ps,
                            func=AF.Identity,
                            bias=b_sb,
                            scale=1.0,
                        )

                # v_aug [128(t), 16*65]: per t-chunk 64 v columns + a ones column
                v_aug = v_pool.tile([128, 16 * (DK + 1)], BF16, tag="vaug")
                nc.vector.memset(v_aug, 1.0)
                v_view = v_aug[:].rearrange("p (t c) -> p t c", c=DK + 1)
                for half in range(2):
                    psv = psc.tile([128, 512], F32, tag="sc")
                    for t8 in range(8):
                        t = 8 * half + t8
                        for d in range(4):
                            nc.tensor.matmul(
                                psv[:, DK * t8 : DK * (t8 + 1)],
                                lhsT=xt[d][:, 128 * t : 128 * (t + 1)],
                                rhs=wv_sb[:, DK * d : DK * (d + 1)],
                                start=(d == 0),
                                stop=(d == 3),
                            )
                    nc.vector.tensor_tensor(
                        out=v_view[:, 8 * half : 8 * half + 8, 0:DK],
                        in0=psv[:].rearrange("p (t c) -> p t c", c=DK),
                        in1=bv8[:].rearrange("p (t c) -> p t c", c=DK),
                        op=ALU.add,
                    )

                # ---- attention: scoresT -> exp -> o accumulation ----
                # Software-pipelined: the o-matmuls for chunk t-1 are emitted
                # after the exp of chunk t, so the PE streams scores(t) and
                # o(t-1) back-to-back while ACT computes exp(t).
                o_ps = [pacc.tile([DK + 1, 512], F32, tag="acc", name=f"ops{b}_{i}") for i in range(4)]
                et_prev = None

                def emit_o(t, et_t):
                    for i in range(4):
                        nc.tensor.matmul(
                            o_ps[i],
                            lhsT=v_view[:, t, :],
                            rhs=et_t[:, 512 * i : 512 * (i + 1)],
                            start=(t == 0),
                            stop=(t == 15),
                        )

                for t in range(16):
                    et = exp_pool.tile([128, S], BF16, tag="et")
                    for half in range(2):
                        ps_sc = psc.tile([128, 1024], F32, tag="sc")
                        for sq in range(2):
                            s4 = 2 * half + sq
                            nc.tensor.matmul(
                                ps_sc[:, 512 * sq : 512 * (sq + 1)],
                                lhsT=kT[:, 128 * t : 128 * (t + 1)],
                                rhs=qT[:, 512 * s4 : 512 * (s4 + 1)],
                                start=True,
                                stop=True,
                            )
                        nc.scalar.activation(
                            out=et[:, 1024 * half : 1024 * (half + 1)],
                            in_=ps_sc,
                            func=AF.Exp,
                            bias=0.0,
                            scale=0.125,
                        )
                    if et_prev is not None:
                        emit_o(t - 1, et_prev)
                    et_prev = et
                emit_o(15, et_prev)

                # ---- normalize (divide by denom row) and ship to send buffer ----
                for i in range(4):
                    rec = small.tile([1, 512], F32R, tag="rec")
                    with nc.allow_low_precision(reason="fp32r softmax recip feeds fp32r matmul"):
                        nc.vector.reciprocal(rec, o_ps[i][DK : DK + 1, :])
                    ps_bc = psc.tile([DK, 512], F32, tag="sc")
                    nc.tensor.matmul(ps_bc, lhsT=ones1, rhs=rec, start=True, stop=True)
                    recb = ot_pool.tile([DK, 512], F32, tag="recb")
                    nc.scalar.copy(recb, ps_bc)
                    ot = ot_pool.tile([DK, 512], F32R, tag="ot")
                    nc.vector.tensor_tensor(
                        out=ot, in0=o_ps[i][0:DK, :], in1=recb, op=ALU.mult
                    )
                    j = 4 * b + i
                    nc.sync.dma_start(
                        out=send2d[16 * j : 16 * (j + 1), :].rearrange(
                            "r (a c) -> (r a) c", a=4
                        ),
                        in_=ot,
                    )

            # =========== exchange heads -> token shards ===========
            nc.gpsimd.collective_compute(
                "AllToAll",
                ALU.bypass,
                replica_groups=[list(range(N_CORES))],
                ins=[send2d[:].opt()],
                outs=[recv2d[:].opt()],
            )

            # o_catT tiles [128(hk), 512(s_local)]
            oc = []
            for cp in range(4):
                t_ = tmp_pool.tile([128, 512], F32R, tag="oc", bufs=4)
                nc.sync.dma_start(
                    out=t_,
                    in_=recv2d[32 * cp : 32 * (cp + 1), :].rearrange(
                        "(j r) (a c) -> (j r a) c", j=2, a=4
                    ),
                )
                oc.append(t_)

            def layernorm(dst, src, g_t, be_t):
                st = small.tile([128, 6], F32, tag="st")
                nc.vector.bn_stats(st, src)
                mv = small.tile([128, 2], F32, tag="mv")
                nc.vector.bn_aggr(mv, st)
                rstd = small.tile([128, 1], F32, tag="rstd")
                nc.scalar.activation(
                    out=rstd, in_=mv[:, 1:2], func=AF.Sqrt, bias=eps_t, scale=1.0
                )
                nc.vector.reciprocal(rstd, rstd)
                tn = tmp_pool.tile([128, D], F32, tag="tn")
                nc.vector.tensor_scalar(
                    out=tn,
                    in0=src,
                    scalar1=mv[:, 0:1],
                    scalar2=rstd,
                    op0=ALU.subtract,
                    op1=ALU.mult,
                )
                tg = tmp_pool.tile([128, D], F32, tag="tg")
                nc.vector.tensor_tensor(out=tg, in0=tn, in1=g_t, op=ALU.mult)
                nc.vector.tensor_tensor(out=dst, in0=tg, in1=be_t, op=ALU.add)

            # ---- Wo projection + residual + LN1 -> h ----
            h = []
            for i in range(4):
                ps_wo = psc.tile([128, 512], F32, tag="sc")
                for cp in range(4):
                    nc.tensor.matmul(
                        ps_wo,
                        lhsT=oc[cp][:, 128 * i : 128 * (i + 1)],
                        rhs=wo_sb[:, 512 * cp : 512 * (cp + 1)],
                        start=(cp == 0),
                        stop=(cp == 3),
                    )
                t1 = h_pool.tile([128, D], F32, tag="t1")
                nc.vector.tensor_tensor(out=t1, in0=ps_wo, in1=xsbo[i], op=ALU.add)
                h_i = h_pool.tile([128, D], F32, tag="h", bufs=4)
                layernorm(h_i, t1, g1_t, be1_t)
                h.append(h_i)

            # ---- hT via PE transpose ----
            hT = [h_pool.tile([128, 512], F32R, tag="hT", name=f"hT{d}", bufs=4) for d in range(4)]
            for i in range(4):
                for d in range(4):
                    ps_t = psc.tile([128, 128], F32, tag="sc")
                    nc.tensor.transpose(
                        ps_t, h[i][:, 128 * d : 128 * (d + 1)], ident
                    )
                    nc.vector.tensor_copy(hT[d][:, 128 * i : 128 * (i + 1)], ps_t)

            # ---- FFN ----
            ff2_ps = [pacc.tile([128, 512], F32, tag="acc", name=f"ff2ps{i}") for i in range(4)]
            for f in range(16):
                w1t = wff_pool.tile([128, 512], F32R, tag="w1t")
                nc.sync.dma_start(
                    out=w1t[:].rearrange("p (d c) -> p d c", c=128),
                    in_=w1[:, 128 * f : 128 * (f + 1)].rearrange(
                        "(d p) c -> p d c", p=128
                    ),
                )
                w2t = wff_pool.tile([128, 512], F32R, tag="w2t")
                nc.sync.dma_start(out=w2t, in_=w2[128 * f : 128 * (f + 1), :])
                ps1 = psc.tile([128, 512], F32, tag="sc")
                for d in range(4):
                    nc.tensor.matmul(
                        ps1,
                        lhsT=w1t[:, 128 * d : 128 * (d + 1)],
                        rhs=hT[d],
                        start=(d == 0),
                        stop=(d == 3),
                    )
                f1 = f1_pool.tile([128, 512], F32R, tag="f1")
                nc.scalar.activation(
                    out=f1, in_=ps1, func=AF.Relu, bias=b1_sb[:, f : f + 1], scale=1.0
                )
                for i in range(4):
                    nc.tensor.matmul(
                        ff2_ps[i],
                        lhsT=f1[:, 128 * i : 128 * (i + 1)],
                        rhs=w2t,
                        start=(f == 0),
                        stop=(f == 15),
                    )

            # ---- epilogue: +b2, +h residual, LN2, store ----
            for i in range(4):
                t1 = tmp_pool.tile([128, D], F32, tag="e1")
                nc.vector.tensor_tensor(out=t1, in0=ff2_ps[i], in1=b2_t, op=ALU.add)
                t2 = tmp_pool.tile([128, D], F32, tag="e2")
                nc.vector.tensor_tensor(out=t2, in0=t1, in1=h[i], op=ALU.add)
                o_sb = tmp_pool.tile([128, D], F32, tag="osb")
                layernorm(o_sb, t2, g2_t, be2_t)
                nc.sync.dma_start(
                    out=out_shard[128 * i : 128 * (i + 1), :], in_=o_sb
                )

    _split_excess_waits(nc)
    return nc


_NC_CACHE = {}


def _get_nc():
    if "nc" not in _NC_CACHE:
        _NC_CACHE["nc"] = build_nc()
    return _NC_CACHE["nc"]


def build_in_maps(inputs):
    x = np.asarray(inputs["x"], np.float32)
    Wq = np.asarray(inputs["Wq"], np.float32)
    bq = np.asarray(inputs["bq"], np.float32)
    Wk = np.asarray(inputs["Wk"], np.float32)
    bk = np.asarray(inputs["bk"], np.float32)
    Wv = np.asarray(inputs["Wv"], np.float32)
    bv = np.asarray(inputs["bv"], np.float32)
    Wo = np.asarray(inputs["Wo"], np.float32)
    bo = np.asarray(inputs["bo"], np.float32)
    ln1_g = np.asarray(inputs["ln1_g"], np.float32)
    ln1_b = np.asarray(inputs["ln1_b"], np.float32)
    W1 = np.asarray(inputs["W1"], np.float32)
    b1 = np.asarray(inputs["b1"], np.float32)
    W2 = np.asarray(inputs["W2"], np.float32)
    b2 = np.asarray(inputs["b2"], np.float32)
    ln2_g = np.asarray(inputs["ln2_g"], np.float32)
    ln2_b = np.asarray(inputs["ln2_b"], np.float32)

    xT = np.ascontiguousarray(x.transpose(0, 2, 1))  # [B, D, S]
    x_flat = x.reshape(B * S, D)
    b1c = np.ascontiguousarray(b1.reshape(DF // 128, 128).T)  # [128, 16]

    in_maps = []
    for c in range(N_CORES):
        in_maps.append(
            {
                "xT": xT,
                "xsb": np.ascontiguousarray(
                    x_flat[NSH * c : NSH * (c + 1)] + bo[None, :]
                ),
                "wq": np.ascontiguousarray(Wq[c]),
                "wk": np.ascontiguousarray(Wk[c]),
                "wv": np.ascontiguousarray(Wv[c]),
                "bq": np.ascontiguousarray(bq[c].reshape(DK, 1)),
                "bk": np.ascontiguousarray(bk[c].reshape(DK, 1)),
                "bv": np.ascontiguousarray(bv[c]),
                "wo": Wo,
                "w1": W1,
                "b1c": b1c,
                "w2": W2,
                "b2": b2,
                "g1": ln1_g,
                "be1": ln1_b,
                "g2": ln2_g,
                "be2": ln2_b,
            }
        )

    return in_maps


def kernel(**inputs):
    in_maps = build_in_maps(inputs)
    nc = _get_nc()
    res = run_bass_kernel_spmd(nc, in_maps, core_ids=list(range(N_CORES)))
    shards = [res.results[c]["out_shard"] for c in range(N_CORES)]
    return np.concatenate(shards, axis=0).reshape(B, S, D)
